# revision 45
# baseline (speedup 1.0000x reference)
"""Temporal attention kernel for Trainium2, data-parallel over batch on 8 cores.

Reference computation (B=64, T=256, D=128, H=8, E=128):
    Q = x@Wq + bq; K = x@Wk + bk; V = x@Wv + bv          [B,T,H,E]
    scores  = einsum('bthd,bjhd->bhtj', Q, K)            [B,H,T,T]
    summary = (scale*scores) @ Ws + bs                   [B,H,T,1]
    beta    = softmax(summary, axis=t)                   [B,H,T]
    result  = sum_t V[b,t,h,:] * beta[b,h,t]             [B,H,E]
    out     = result.reshape(B,H*E) @ Wo + bo            [B,D]

Algebraic restructure (exact up to fp reassociation):
  * Ws contracts the key axis immediately, so K enters only through
    xs_b = x_b^T Ws, and the logits are
      summary[t,h] = x_b[t,:] @ (M_h xs_b + sws*q0_h)
    with M_h = scale*Wq_h Wk_h^T and q0_h = scale*Wq_h bk_h folded on host.
  * softmax over t is shift-invariant => bq/bs terms drop; logits are O(0.05)
    so exp() without max-subtraction is exact.
  * V and the output projection collapse: out = sum_h N_h^T xbt_h + b0 with
    N_h = Wv_h Wo_h and b0 = bo + Wo^T bv folded on host (b0 added on host).
  * beta normalization: sum_t exp is materialized pre-broadcast down all 128
    partitions by a ones-matrix matmul (esumB = 1s^T E), reciprocated on DVE,
    and applied as one elementwise multiply against the PSUM-resident
    weighted V-sum - no transposes anywhere in the kernel.
  Precision: everything DMA'd travels fp8e4m3 except N (bf16) and the fp32
  output. x ships as hi + residual-lo fp8 pairs (hi+lo ~ bf16 accuracy; the
  logits path consumes hi only - logits are O(0.05) and softmax is
  shift-tolerant). M/Ws/q0 are pre-scaled on host (256/64/4096) against fp8
  underflow and compensated on-chip. Rel err ~2e-3 << 2e-2 tol.
  x also ships in [d,.,t] fp8 layout so no on-chip transposes are needed.
  DMAs spread across the three DMA-capable queues (SP / Pool / Act) in
  first-use order; the two sample groups are emitted stage-interleaved so no
  engine's in-order queue head-of-line-blocks the other group.
"""

import contextlib

import numpy as np

import concourse.bacc as bacc
import concourse.mybir as mybir
import concourse.tile as tile
from concourse.bass_utils import run_bass_kernel_spmd

N_CORES = 8
B, T, D = 64, 256, 128
H, E = 8, 128
HE = H * E
BL = B // N_CORES          # samples per core (8)
TC = T // 128              # 128-token chunks per sample (2)
NJ = BL * TC               # token chunks per core (16)
NG = 2                     # sample groups per core (pipelining)
GBS = [4, 4]               # samples per group (equal split measured fastest)
GB0 = GBS[0]
HJS = [g * TC for g in GBS]        # chunks per group
GHS = [g * H for g in GBS]         # (b,h) pairs per group
SCALE = 1.0 / float(np.sqrt(np.float32(E)))
MSCALE = 256.0             # fp8 underflow guard on M
WSCALE = 64.0              # fp8 underflow guard on Ws
QSCALE = 4096.0            # fp8 underflow guard on q0

FP32 = mybir.dt.float32
BF16 = mybir.dt.bfloat16
FP8 = mybir.dt.float8e4
NP_BF16 = mybir.dt.np(BF16)
NP_FP8 = mybir.dt.np(FP8)
AF = mybir.ActivationFunctionType

XW0 = GBS[0] * TC * 128    # x cols group 0
XW1 = GBS[1] * TC * 128    # x cols group 1
C_WS = XW0                 # Ws*WSCALE chunks (TC cols) appended to xh0
C_Q0 = C_WS + TC           # q0*QSCALE, H cols

_cached = {}


def _build_program():
    nc = bacc.Bacc("TRN2", target_bir_lowering=False, debug=False)

    xh0_d = nc.dram_tensor("xh0", [128, XW0 + TC + H], FP8,
                           kind="ExternalInput").ap()
    xh1_d = nc.dram_tensor("xh1", [128, XW1], FP8, kind="ExternalInput").ap()
    xl_d = nc.dram_tensor("xl", [128, NJ, 128], FP8,
                          kind="ExternalInput").ap()
    xt_d = nc.dram_tensor("xt", [128, NJ, 128], FP8, kind="ExternalInput").ap()
    mt_d = nc.dram_tensor("mt", [128, H * 128], FP8, kind="ExternalInput").ap()
    nn_d = nc.dram_tensor("nn", [128, H * 128], BF16, kind="ExternalInput").ap()
    y_d = nc.dram_tensor("y", [D, BL], FP32, kind="ExternalOutput").ap()

    with tile.TileContext(nc) as tc:
        _emit(tc, xh0_d, xh1_d, xl_d, xt_d, mt_d, nn_d, y_d)
    nc.compile()
    return nc


def _emit(tc, xh0_d, xh1_d, xl_d, xt_d, mt_d, nn_d, y_d):
    nc = tc.nc
    with contextlib.ExitStack() as ctx:
        cpool = ctx.enter_context(tc.tile_pool(name="consts", bufs=1))
        ppool = ctx.enter_context(tc.tile_pool(name="psums", bufs=1,
                                               space="PSUM"))

        # ---- persistent SBUF tiles ----
        xh0_sb = cpool.tile([128, XW0 + TC + H], FP8, tag="xh0")
        xh1_sb = cpool.tile([128, XW1], FP8, tag="xh1")
        xl_sb = [cpool.tile([128, HJS[g], 128], FP8, tag=f"xl{g}",
                            name=f"xl{g}") for g in range(NG)]
        xt_sb = [cpool.tile([128, HJS[g], 128], FP8, tag=f"xt{g}",
                            name=f"xt{g}") for g in range(NG)]
        mt_sb = cpool.tile([128, H * 128], FP8, tag="mt")
        nn_sb = cpool.tile([128, H * 128], BF16, tag="nn")
        onem_sb = cpool.tile([128, 128], BF16, tag="onem")  # ones matrix
        cxs_sb = cpool.tile([128, 1], FP32, tag="cxs")      # 1/(MSCALE*WSCALE)
        cq_sb = cpool.tile([128, 1], FP32, tag="cq")        # 1/QSCALE
        q0f_sb = cpool.tile([128, H], FP32, tag="q0f")      # q0 upcast
        xst_sb = cpool.tile([128, BL], BF16, tag="xst")     # [d, b]
        wqh_sb = cpool.tile([128, H, BL], BF16, tag="wqh")  # [d, h, b]
        e_sb = cpool.tile([128, TC, BL, H], BF16, tag="esb")  # [t, c, b, h]
        recb_sb = [cpool.tile([128, GHS[g]], FP32, tag=f"rb{g}",
                              name=f"rb{g}") for g in range(NG)]
        xbt_sb = cpool.tile([128, BL, H], BF16, tag="xbt")  # normalized
        outt_sb = cpool.tile([128, BL], FP32, tag="outt")   # [dout, b]

        xh_g = [xh0_sb[:, :XW0].rearrange("t (j d) -> t j d", d=128),
                xh1_sb.rearrange("t (j d) -> t j d", d=128)]
        ws_col = [xh0_sb[:, C_WS + c:C_WS + c + 1] for c in range(TC)]
        q08 = xh0_sb[:, C_Q0:C_Q0 + H]

        # ---- input DMAs across the three DMA-capable queues ----
        # SP: xh0(+ws,q0), MT, xlo0, xlo1;  Pool: xh1, xT0, xT1;  Act: NN
        nc.sync.dma_start(xh0_sb[:], xh0_d)
        nc.sync.dma_start(mt_sb[:], mt_d)
        HJ0 = HJS[0]
        nc.sync.dma_start(xl_sb[0][:], xl_d[:, :HJ0, :])
        nc.sync.dma_start(xl_sb[1][:], xl_d[:, HJ0:, :])
        nc.gpsimd.dma_start(xh1_sb[:], xh1_d)
        nc.gpsimd.dma_start(xt_sb[0][:], xt_d[:, :HJ0, :])
        nc.gpsimd.dma_start(xt_sb[1][:], xt_d[:, HJ0:, :])
        nc.scalar.dma_start(nn_sb[:], nn_d)

        # constants built on idle DVE lanes at t~0
        nc.vector.memset(onem_sb[:], 1.0)
        nc.vector.memset(cxs_sb[:], 1.0 / (MSCALE * WSCALE))
        nc.vector.memset(cq_sb[:], 1.0 / QSCALE)

        # ---- shared PSUM tiles; groups use disjoint regions ----
        ps_xe = ppool.tile([128, 8], FP32, tag="xe")        # xst cols
        ps_wqh = ppool.tile([128, H, BL], FP32, tag="wqh")
        ps_summ = ppool.tile([128, TC, BL, H], FP32, tag="summ")
        ps_xbtu = ppool.tile([128, BL, H], FP32, tag="xbtu")
        ps_rb = ppool.tile([128, NG, GHS[0]], FP32, tag="rb")  # esum bcast
        ps_out = ppool.tile([128, BL], FP32, tag="outp")

        def bounds(g):
            b0 = sum(GBS[:g])
            return b0, b0 + GBS[g]

        def xst(g):
            b0, b1 = bounds(g)
            for b in range(b0, b1):
                for c in range(TC):
                    jl = (b - b0) * TC + c
                    nc.tensor.matmul(ps_xe[:, b:b + 1], xh_g[g][:, jl, :],
                                     ws_col[c],
                                     start=(c == 0), stop=(c == TC - 1))
            if g == 0:
                nc.vector.tensor_scalar_mul(xst_sb[:, b0:b1],
                                            ps_xe[:, b0:b1], cxs_sb[:])
            else:
                nc.scalar.activation(xst_sb[:, b0:b1], ps_xe[:, b0:b1],
                                     AF.Copy, scale=cxs_sb[:, 0:1])

        def wqh(g):
            b0, b1 = bounds(g)
            for h in range(H):
                nc.tensor.matmul(ps_wqh[:, h, b0:b1],
                                 mt_sb[:, h * 128:(h + 1) * 128],
                                 xst_sb[:, b0:b1], start=True, stop=True)
            nc.vector.tensor_add(
                wqh_sb[:, :, b0:b1], ps_wqh[:, :, b0:b1],
                q0f_sb[:, :, None].broadcast_to([128, H, b1 - b0]))

        def summ(g):
            b0, b1 = bounds(g)
            for b in range(b0, b1):
                for c in range(TC):
                    jl = (b - b0) * TC + c
                    nc.tensor.matmul(ps_summ[:, c, b, :],
                                     xt_sb[g][:, jl, :], wqh_sb[:, :, b],
                                     start=True, stop=True)
            nc.scalar.activation(e_sb[:, :, b0:b1, :], ps_summ[:, :, b0:b1, :],
                                 AF.Exp)

        def esum(g):
            b0, b1 = bounds(g)
            for c in range(TC):
                nc.tensor.matmul(
                    ps_rb[:, g, :GHS[g]], onem_sb[:],
                    e_sb[:, c, b0:b1, :].rearrange("t b h -> t (b h)"),
                    start=(c == 0), stop=(c == TC - 1))
            nc.vector.reciprocal(recb_sb[g][:], ps_rb[:, g, :GHS[g]])

        def xbtu(g):
            b0, b1 = bounds(g)
            for b in range(b0, b1):
                for c in range(TC):
                    jl = (b - b0) * TC + c
                    nc.tensor.matmul(ps_xbtu[:, b, :], xh_g[g][:, jl, :],
                                     e_sb[:, c, b, :],
                                     start=(c == 0), stop=False)
                    nc.tensor.matmul(ps_xbtu[:, b, :], xl_sb[g][:, jl, :],
                                     e_sb[:, c, b, :],
                                     start=False, stop=(c == TC - 1))

        def recb(g):
            b0, b1 = bounds(g)
            nc.vector.tensor_mul(
                xbt_sb[:, b0:b1, :].rearrange("d b h -> d (b h)"),
                ps_xbtu[:, b0:b1, :].rearrange("d b h -> d (b h)"),
                recb_sb[g][:])

        def back(g):
            b0, b1 = bounds(g)
            for h in range(H):
                nc.tensor.matmul(ps_out[:, b0:b1],
                                 nn_sb[:, h * 128:(h + 1) * 128],
                                 xbt_sb[:, b0:b1, h],
                                 start=(h == 0), stop=(h == H - 1))
            nc.scalar.copy(outt_sb[:, b0:b1], ps_out[:, b0:b1])

        # emission = per-engine program order; staged so no group's op
        # head-of-line-blocks the other group's earlier stage
        xst(0)
        nc.vector.tensor_scalar_mul(q0f_sb[:], q08, cq_sb[:])
        xst(1)
        wqh(0)
        wqh(1)
        summ(0)
        summ(1)
        esum(0)
        xbtu(0)
        recb(0)
        esum(1)
        xbtu(1)
        recb(1)
        back(0)
        back(1)

        # ---- y[dout, b]; host transposes and adds b0 ----
        nc.scalar.dma_start(y_d, outt_sb[:])


def _prep_in_maps(inputs):
    x = np.asarray(inputs["x"], dtype=np.float32)
    Wq = np.asarray(inputs["Wq"], dtype=np.float32)
    Wk = np.asarray(inputs["Wk"], dtype=np.float32)
    Wv = np.asarray(inputs["Wv"], dtype=np.float32)
    Wo = np.asarray(inputs["Wo"], dtype=np.float32)
    bk = np.asarray(inputs["bk"], dtype=np.float32)
    Ws = np.asarray(inputs["Ws"], dtype=np.float32).reshape(T)
    sws = float(Ws.sum())

    mt = np.zeros((128, H * 128), dtype=np.float32)
    nn = np.zeros((128, H * 128), dtype=np.float32)
    q0 = np.zeros((128, H), dtype=np.float32)
    for h in range(H):
        Wq_h = Wq[:, h * E:(h + 1) * E]
        Wk_h = Wk[:, h * E:(h + 1) * E]
        Wv_h = Wv[:, h * E:(h + 1) * E]
        Wo_h = Wo[h * E:(h + 1) * E, :]
        q0[:, h] = (QSCALE * sws * SCALE) * (Wq_h @ bk[h * E:(h + 1) * E])
        mt[:, h * 128:(h + 1) * 128] = (MSCALE * SCALE) * (Wk_h @ Wq_h.T)
        nn[:, h * 128:(h + 1) * 128] = Wv_h @ Wo_h

    shared = {"mt": mt.astype(NP_FP8), "nn": nn.astype(NP_BF16)}
    cargo = np.concatenate(
        [(WSCALE * Ws).reshape(TC, 128).T, q0], axis=1)
    in_maps = []
    for core in range(N_CORES):
        xc = x[core * BL:(core + 1) * BL]                  # [BL, T, D]
        xq = xc.reshape(BL, TC, 128, D)
        xr = xq.transpose(2, 0, 1, 3).reshape(128, NJ * D)   # [t, (b c d)]
        xtr = xq.transpose(3, 0, 1, 2).reshape(128, NJ * 128)  # [d, (b c t)]
        xhi = xr.astype(NP_FP8)
        xlo = (xr - xhi.astype(np.float32)).astype(NP_FP8)
        xh0 = np.concatenate(
            [xhi[:, :XW0].astype(np.float32), cargo], axis=1)
        in_maps.append({
            "xh0": np.ascontiguousarray(xh0.astype(NP_FP8)),
            "xh1": np.ascontiguousarray(xhi[:, XW0:]),
            "xl": np.ascontiguousarray(xlo).reshape(128, NJ, D),
            "xt": np.ascontiguousarray(xtr.astype(NP_FP8)).reshape(
                128, NJ, 128),
            **shared,
        })
    return in_maps


def kernel(**inputs):
    if "nc" not in _cached:
        _cached["nc"] = _build_program()
    nc = _cached["nc"]
    in_maps = _prep_in_maps(inputs)
    res = run_bass_kernel_spmd(nc, in_maps, list(range(N_CORES)))
    _cached["last_results"] = res

    Wo = np.asarray(inputs["Wo"], dtype=np.float32)
    bv = np.asarray(inputs["bv"], dtype=np.float32)
    bo = np.asarray(inputs["bo"], dtype=np.float32)
    b0 = bo + bv @ Wo
    return np.concatenate(
        [res.results[c]["y"].T + b0 for c in range(N_CORES)], axis=0
    ).astype(np.float32)
